# revision 50
# baseline (speedup 1.0000x reference)
"""Multi-head causal attention (B=16, T=512, D=1024, H=16) on 8 TRN2 cores.

Sharding: data-parallel over batch (2 batches per core), weights replicated.

Design (vs the previous kernel):
- Weights are loaded from HBM ONCE per invocation into reduced-precision
  resident SBUF tiles (fp8e4 DoubleRow layout for Q/K via SWDGE cast-DMA,
  bf16 for V/W_O): HBM traffic 36MB -> 24MB, and the b1 phases run with
  zero weight DMA.
- Q/K projections are fp8e4 DoubleRow matmuls (2x contraction per cycle;
  HW-validated ~2x vs bf16, rel err 4.3e-3 vs the 2e-2 gate -- softmax
  damps Q/K quantization; V/W_O in fp8 would fail at ~3.8e-2, so those
  stay bf16).
- Attention processes HEAD PAIRS (heads 2j/2j+1 on feature partitions
  0-63/64-127): the K=64 S^T matmuls alternate PE row groups so the two
  heads' matmuls run concurrently (HW-validated 2.28x incl. LDWEIGHTS
  overlap). The pair's 8 causal chunks pack into 5 PSUM banks
  (b0:A-kt0 | b1:A-kt1+A-kt3 | b2:A-kt2+B-kt2 | b3:B-kt0 | b4:B-kt1+B-kt3).
- HW hazard found by bisection: back-to-back matmul groups into the SAME
  psum bank with K=64 row-tiled closers crash the device (group-boundary
  bank revisit). Rule: separate same-bank groups with other-bank matmuls
  AND close shared-bank groups with a full-row (K=128) matmul. The causal
  mask matmuls (ident @ maskT, -1e30 additive) serve as those closers for
  kt1/kt2/kt3; kt0 (unshared, single-group) is masked post-exp by a DVE
  lower-triangular multiply.
- exp(S/32) evacuates per bank on ACT (5 ops/pair) so the next pair's S
  matmuls unblock per-bank instead of waiting a fused whole-tile read.
- Softmax normalize: ones column in V gives row sums in psy[64]; psy is
  evacuated to SBUF (bf16) immediately after AV so the psum bank frees
  after two fast DVE reads (copy + reciprocal) instead of being held
  through broadcast+normalize; gpsimd partition_broadcast of the
  reciprocal row, then a 2x-rate bf16 DVE multiply into y_t.
- Schedule: both batches' x^T transposes run at startup under the weight
  DMA; Q(b1)+V(b1) fill PE during attention(b0) (one full-row-closed
  group per pair slot in the shared filler bank); K(b1) between phases;
  full W_O chunks fill attention(b1); remaining W_O chunks drain in the
  tail rotating over 3 psum banks. Each batch's S(pair0) is pre-emitted
  as soon as its fo=0 q/k blocks exist (before the V section / rest of
  K), so the first exp overlaps the remaining projection matmuls.
Measured (same-day, conditions-matched differential bench): 135-204us
median depending on device contention (min-quartile 81us in quiet
windows) vs 258-280us for the previous kernel, rel err 4.1e-3.
"""

import os
import sys

sys.path.insert(0, "/opt/trn_rl_repo")

import numpy as np

QK_DR = os.environ.get("QK_DR", "1") == "1"
CAST_DMA = os.environ.get("CAST_DMA", "1") == "1"
FUSED_EXP = os.environ.get("FUSED_EXP", "0") == "1"
MULTIBANK = os.environ.get("MULTIBANK", "0") == "1"
MASKS = os.environ.get("MASKS", "1") == "1"
PHASES = int(os.environ.get("PHASES", "4"))

B, T, D = 16, 512, 1024
H = 16
HD = D // H          # 64
NCORES = 8
BL = B // NCORES     # 2 local batches per core
PPART = 128
VW = HD + 1          # 65: head dim + ones column for row sums

_CACHE = {}


def _build_program(reps=1):
    import concourse.bass as bass
    import concourse.tile as tile
    from concourse import bacc, mybir
    from concourse.masks import make_identity

    DT = mybir.dt.float32
    R = mybir.dt.float32r
    BF = mybir.dt.bfloat16
    F8 = mybir.dt.float8e4
    ACTF = mybir.ActivationFunctionType
    DR = mybir.MatmulPerfMode.DoubleRow

    nc = bacc.Bacc("TRN2", target_bir_lowering=False, debug=False,
                   num_devices=NCORES)

    x_d = nc.dram_tensor("x", [BL, T, D], DT, kind="ExternalInput").ap()
    wqkv_d = nc.dram_tensor("w_qkv", [D, 3 * D], DT, kind="ExternalInput").ap()
    bqkv_d = nc.dram_tensor("b_qkv", [3 * D], DT, kind="ExternalInput").ap()
    wo_d = nc.dram_tensor("w_o", [D, D], DT, kind="ExternalInput").ap()
    bo_d = nc.dram_tensor("b_o", [D], DT, kind="ExternalInput").ap()
    out_d = nc.dram_tensor("out", [BL, T, D], DT, kind="ExternalOutput").ap()

    x_f = x_d.flatten_outer_dims()      # [1024, 1024] tokens x features
    out_fs = [out_d.flatten_outer_dims()]
    for r in range(1, reps):
        scr = nc.dram_tensor(f"scratch{r}", [BL, T, D], DT).ap()
        out_fs.append(scr.flatten_outer_dims())

    def f32r(ap):
        return ap.bitcast(R)

    with tile.TileContext(nc) as tc:
        with (
            tc.tile_pool(name="consts", bufs=1) as consts,
            tc.tile_pool(name="w8", bufs=1) as w8_pool,
            tc.tile_pool(name="wvb", bufs=1) as wv_pool,
            tc.tile_pool(name="wob", bufs=1) as wo_pool,
            tc.tile_pool(name="y", bufs=1) as y_pool,
            tc.tile_pool(name="xb", bufs=1) as xb_pool,
            tc.tile_pool(name="x8", bufs=1) as x8_pool,
            tc.tile_pool(name="qkv", bufs=1) as qkv_pool,
            tc.tile_pool(name="xn", bufs=4) as xn_pool,
            tc.tile_pool(name="pp", bufs=3) as p_pool,
            tc.tile_pool(name="ss", bufs=4) as s_pool,
            tc.tile_pool(name="ob", bufs=3) as o_pool,
        ):
            # ---------------- constants ----------------
            ident_f = consts.tile([PPART, PPART], DT)
            make_identity(nc, ident_f)
            ident = consts.tile([PPART, PPART], R)
            nc.vector.tensor_copy(out=ident, in_=ident_f)

            # lower-tri 0/1 bf16 mask for post-exp causal zeroing of the
            # diagonal blocks of P^T: entry (k=p, q=c) kept iff c >= p
            trimask_f = consts.tile([PPART, PPART], DT)
            nc.vector.memset(trimask_f, 1.0)
            nc.gpsimd.affine_select(
                out=trimask_f, in_=trimask_f,
                compare_op=mybir.AluOpType.is_ge, fill=0.0,
                base=0, pattern=[[1, PPART]], channel_multiplier=-1,
            )
            trimask = consts.tile([PPART, PPART], BF)
            nc.vector.tensor_copy(out=trimask, in_=trimask_f)

            # additive causal mask for the PE group-closer matmuls:
            # (k=i, q=j) kept (0.0) where j - i >= 0, else -1e30
            maskneg = consts.tile([PPART, PPART], DT)
            nc.vector.memset(maskneg, 0.0)
            nc.gpsimd.affine_select(
                out=maskneg, in_=maskneg,
                compare_op=mybir.AluOpType.is_ge, fill=-1e30,
                base=0, pattern=[[1, PPART]], channel_multiplier=-1,
            )
            maskT_b = consts.tile([PPART, PPART], BF)
            nc.vector.tensor_copy(out=maskT_b, in_=maskneg)
            ident_b = consts.tile([PPART, PPART], BF)
            nc.vector.tensor_copy(out=ident_b, in_=ident_f)
            zcol = consts.tile([PPART, 1], BF)
            nc.vector.memset(zcol, 0.0)

            # qkv bias, feature-major columns: bcol[p, c] = b_qkv[128c + p]
            bcol = consts.tile([PPART, 24], DT)
            brow = consts.tile([1, D], DT)
            bv_bc = consts.tile([PPART, D], DT)
            bo_bc = consts.tile([PPART, D], DT)

            def load_biases():
                nc.sync.dma_start(
                    out=bcol, in_=bqkv_d.rearrange("(c p) -> p c", p=PPART))
                nc.sync.dma_start(
                    out=brow,
                    in_=bqkv_d[2 * D:3 * D].rearrange("(a f) -> a f", a=1))
                nc.gpsimd.partition_broadcast(bv_bc, brow, channels=PPART)
                nc.sync.dma_start(
                    out=brow, in_=bo_d.rearrange("(a f) -> a f", a=1))
                nc.gpsimd.partition_broadcast(bo_bc, brow, channels=PPART)

            # ---------------- resident weights ----------------
            # fp8e4 DoubleRow layout for Q,K: w8[sec][c][p, i, f] =
            # w_qkv[256c + 128i + p, 1024 sec + f]   (unscaled; validated)
            w8 = {}

            def _cast_load(dst, src_ap, flat=None):
                # HBM fp32 -> reduced-precision SBUF tile: SWDGE cast DMA,
                # or (fallback) HWDGE fp32 stage + DVE convert
                if CAST_DMA:
                    nc.gpsimd.dma_start(out=dst, in_=src_ap)
                else:
                    dflat = flat if flat is not None else dst
                    stage = consts.tile([PPART, 2048], DT, tag="wstage",
                                        bufs=1, name="wstage")
                    sz = dflat.free_size()
                    nc.sync.dma_start(out=stage[:, :sz], in_=src_ap)
                    nc.vector.tensor_copy(out=dflat, in_=stage[:, :sz])

            def load_w8(sec):
                tiles = []
                for c in range(4):
                    t = w8_pool.tile([PPART, 2, 1024], F8 if QK_DR else BF,
                                     name=f"w8_{sec}_{c}")
                    _cast_load(
                        t,
                        wqkv_d[256 * c:256 * (c + 1),
                               1024 * sec:1024 * (sec + 1)]
                        .rearrange("(i p) f -> p i f", p=PPART),
                        flat=t.rearrange("p a b -> p (a b)"))
                    tiles.append(t)
                w8[sec] = tiles

            wv_b = []

            def load_wv():
                for ko in range(8):
                    t = wv_pool.tile([PPART, 1024], BF, name=f"wv_{ko}")
                    _cast_load(t, wqkv_d[128 * ko:128 * (ko + 1), 2048:3072])
                    wv_b.append(t)

            wo_b = []

            def load_wo():
                for ko in range(8):
                    t = wo_pool.tile([PPART, 1024], BF, name=f"wo_{ko}")
                    _cast_load(t, wo_d[128 * ko:128 * (ko + 1), :])
                    wo_b.append(t)

            y_t = y_pool.tile([PPART, 8, BL * T], BF)  # [128, 8, 1024]

            # ---------------- x load / transpose / convert ----------------
            def start_xn_dma(b, to, split=False):
                xn = xn_pool.tile([PPART, D], R)
                src = x_f[T * b + 128 * to:T * b + 128 * (to + 1), :]
                if split:
                    for c in range(2):
                        nc.sync.dma_start(
                            out=xn[:, 512 * c:512 * (c + 1)],
                            in_=f32r(src[:, 512 * c:512 * (c + 1)]))
                else:
                    nc.sync.dma_start(out=xn, in_=f32r(src))
                return xn

            def transpose_fg(xb_t, xn, to, fg, trps, x8_t=None):
                # 4 transposes -> one psum tile -> DVE evac to bf16 xb_t
                # (and optionally a second DVE evac to fp8 x8_t for b0)
                pst_d = trps.tile([PPART, 4 * PPART], DT, tag="tr")
                pst = pst_d.bitcast(R).rearrange("p (f q) -> p f q", f=4)
                for fi in range(4):
                    fo = 4 * fg + fi
                    nc.tensor.transpose(
                        pst[:, fi, :], xn[:, 128 * fo:128 * (fo + 1)], ident)
                dst = xb_t[:, 4 * fg:4 * (fg + 1), 128 * to:128 * (to + 1)]
                nc.vector.tensor_copy(out=dst, in_=pst_d)
                if x8_t is not None:
                    dst8 = x8_t[:, 4 * fg:4 * (fg + 1),
                                128 * to:128 * (to + 1)]
                    nc.vector.tensor_copy(out=dst8, in_=pst_d)

            def x8_cast_dma(x8_t, xb_t, c):
                # SWDGE cast-copy bf16 -> fp8 of fo pair block c (b1 path)
                if not QK_DR:
                    return
                if CAST_DMA:
                    nc.gpsimd.dma_start(
                        out=x8_t[:, 2 * c:2 * (c + 1), :],
                        in_=xb_t[:, 2 * c:2 * (c + 1), :])
                else:
                    nc.vector.tensor_copy(
                        out=x8_t[:, 2 * c:2 * (c + 1), :],
                        in_=xb_t[:, 2 * c:2 * (c + 1), :])

            # ---------------- projections ----------------
            def alloc_qkv(which):
                t = {}
                if "q" in which:
                    t["q"] = qkv_pool.tile([PPART, 8, T], BF, tag="q",
                                           name="q_t", bufs=2)
                if "k" in which:
                    t["k"] = qkv_pool.tile([PPART, 8, T], BF, tag="k",
                                           name="k_t", bufs=2)
                if "v" in which:
                    v_t = qkv_pool.tile([PPART, 4, H * VW], BF, tag="v",
                                        bufs=2)
                    ones = v_t.rearrange("p t (h c) -> p t h c",
                                         c=VW)[:, :, :, HD:]
                    nc.vector.memset(ones, 1.0)
                    t["v"] = v_t
                return t

            def qk_fo(sec, x8_t, dst, fo, qps, tag="qkv", bufs=3,
                      evac="act"):
                # one output block of the Q or K projection: 4 DoubleRow
                # matmuls (256-feature chunks) into one psum, ACT evac+bias
                ps = qps.tile([PPART, T], DT, tag=tag, bufs=bufs,
                              name=f"qk{sec}{fo}")
                if QK_DR:
                    for c in range(4):
                        nc.tensor.matmul(
                            ps,
                            lhsT=w8[sec][c][:, :, 128 * fo:128 * (fo + 1)],
                            rhs=x8_t[:, 2 * c:2 * (c + 1), :],
                            perf_mode=DR,
                            start=(c == 0), stop=(c == 3))
                else:
                    for c in range(4):
                        for i in range(2):
                            nc.tensor.matmul(
                                ps,
                                lhsT=w8[sec][c][:, i,
                                                128 * fo:128 * (fo + 1)],
                                rhs=x8_t[:, 2 * c + i, :],
                                start=(c == 0 and i == 0),
                                stop=(c == 3 and i == 1))
                if evac == "act":
                    nc.scalar.activation(
                        out=dst[:, fo, :], in_=ps, func=ACTF.Identity,
                        bias=bcol[:, 8 * sec + fo:8 * sec + fo + 1])
                else:
                    # DVE evac keeps ACT free for the attention exps
                    nc.vector.tensor_scalar_add(
                        out=dst[:, fo, :], in0=ps,
                        scalar1=bcol[:, 8 * sec + fo:8 * sec + fo + 1])

            def v_block(xb_t, v_t, to, nh, qps, tag="qkv", bufs=3):
                # one V psum block: 8 bf16 matmuls, DVE evac + bias
                ps = qps.tile([PPART, T], DT, tag=tag, bufs=bufs,
                              name=f"v{to}{nh}")
                for ko in range(8):
                    nc.tensor.matmul(
                        ps,
                        lhsT=xb_t[:, ko, 128 * to:128 * (to + 1)],
                        rhs=wv_b[ko][:, 512 * nh:512 * (nh + 1)],
                        start=(ko == 0), stop=(ko == 7))
                vv = v_t[:, to, 8 * VW * nh:8 * VW * (nh + 1)]
                vv = vv.rearrange("p (h c) -> p h c", c=VW)[:, :, :HD]
                nc.vector.tensor_add(
                    out=vv, in0=ps, in1=bv_bc[:, 512 * nh:512 * (nh + 1)])

            # --------------- attention (head pairs) ---------------
            # pair j = heads (2j, 2j+1) at feature partitions 0-63 / 64-127.
            # S^T chunks land in one 5-bank psum tile s5 [128, 5, 512]:
            #  bank0: A kt0 (q0-512)   bank1: A kt1 (q128-512) + A kt3
            #  bank2: A kt2 (q256-512) + B kt2    bank3: B kt0
            #  bank4: B kt1 + B kt3
            # -> ONE exp op over 2560 cols -> pch bf16 [128, 2560]
            state = {}
            SL_A = {0: (0, 0, 512), 1: (1, 0, 384), 2: (2, 0, 256),
                    3: (1, 384, 128)}
            SL_B = {0: (3, 0, 512), 1: (4, 0, 384), 2: (2, 256, 256),
                    3: (4, 384, 128)}
            # pch col offsets of each chunk (bank*512 + off)
            PCH_A = {kt: SL_A[kt][0] * 512 + SL_A[kt][1] for kt in range(4)}
            PCH_B = {kt: SL_B[kt][0] * 512 + SL_B[kt][1] for kt in range(4)}
            # diagonal 128-col blocks to mask (pch col offsets)
            DIAG = [PCH_A[0], PCH_A[1], PCH_A[3], PCH_A[2],
                    PCH_B[0], PCH_B[1], PCH_B[3], PCH_B[2]]

            def attn_S(b, j, q_t, k_t, sps, tag="s", bufs=1):
                if MULTIBANK:
                    s5 = sps.tile([PPART, 5, 512], DT, tag=tag, bufs=bufs,
                                  name="s5")
                    banks = [s5[:, i, :] for i in range(5)]
                else:
                    banks = [sps.tile([PPART, 512], DT, tag=tag,
                                      bufs=5 * bufs, name=f"s5_{i}")
                             for i in range(5)]

                def smm(kt, h, start, stop):
                    base = 64 * h
                    bank, off, nq = (SL_A if h == 0 else SL_B)[kt]
                    nc.tensor.matmul(
                        banks[bank][:, off:off + nq],
                        lhsT=k_t[base:base + 64, j,
                                 128 * kt:128 * (kt + 1)],
                        rhs=q_t[base:base + 64, j, 128 * kt:],
                        start=start, stop=stop, skip_group_check=True)

                def mmask(kt, h):
                    # K=128 causal-mask matmul: applies the additive mask
                    # to the chunk's diagonal block AND closes the psum
                    # group (shared-bank groups not closed by a full-row
                    # matmul hazard the PE->PSUM path)
                    bank, off, nq = (SL_A if h == 0 else SL_B)[kt]
                    nc.tensor.matmul(
                        banks[bank][:, off:off + 128],
                        lhsT=ident_b, rhs=maskT_b,
                        start=False, stop=True, skip_group_check=True)

                # order avoids consecutive same-bank writes across group
                # boundaries while alternating A/B row groups for PE
                # concurrency; kt0 (unshared banks) closes immediately
                # and its diagonal is masked post-exp on DVE
                smm(0, 0, True, True)
                smm(0, 1, True, True)
                smm(1, 0, True, False)
                smm(1, 1, True, False)
                mmask(1, 0)
                mmask(1, 1)
                smm(2, 0, True, False)
                mmask(2, 0)
                smm(3, 0, True, False)
                smm(3, 1, True, False)
                mmask(3, 0)
                mmask(3, 1)
                smm(2, 1, True, False)
                mmask(2, 1)
                pch = p_pool.tile([PPART, 5 * 512], BF, tag="P")
                if FUSED_EXP and MULTIBANK:
                    nc.scalar.activation(
                        out=pch, in_=s5.rearrange("p a b -> p (a b)"),
                        func=ACTF.Exp, scale=1.0 / 32.0)
                else:
                    for bank in range(5):
                        nc.scalar.activation(
                            out=pch[:, 512 * bank:512 * (bank + 1)],
                            in_=banks[bank],
                            func=ACTF.Exp, scale=1.0 / 32.0)
                # only the kt0 diagonals need DVE masking (kt1-3 were
                # masked by the PE group-closer matmuls pre-exp)
                for off in (PCH_A[0], PCH_B[0]):
                    nc.vector.tensor_mul(
                        out=pch[:, off:off + 128],
                        in0=pch[:, off:off + 128], in1=trimask)
                state[(b, j)] = pch

            def attn_AV(b, j, v_t, yps):
                pch = state.pop((b, j))
                for h in range(2):
                    hh = 2 * j + h
                    pc = PCH_A if h == 0 else PCH_B
                    psy = yps.tile([PPART, T], DT, tag="y", name=f"psy{h}")
                    for kt in range(4):
                        nc.tensor.matmul(
                            psy[:VW, 128 * kt:],
                            lhsT=v_t[:, kt, VW * hh:VW * (hh + 1)],
                            rhs=pch[:, pc[kt]:pc[kt] + (512 - 128 * kt)],
                            start=(kt == 0), stop=(kt == 3))
                    # evacuate psy to SBUF right away so the psum bank
                    # frees after two fast DVE reads (copy + recip) instead
                    # of being held through broadcast+normalize
                    ybuf = s_pool.tile([HD, T], BF, tag="yb", bufs=4,
                                       name="ybuf")
                    nc.vector.tensor_copy(out=ybuf, in_=psy[:HD, :])
                    r_row = s_pool.tile([1, T], BF, tag="r1")
                    with nc.allow_low_precision(reason="bf16 softmax recip"):
                        nc.vector.reciprocal(r_row, psy[HD:HD + 1, :])
                    r64 = s_pool.tile([HD, T], BF, tag="r64", bufs=4)
                    nc.gpsimd.partition_broadcast(r64, r_row, channels=HD)
                    state[(b, hh, "n")] = (ybuf, r64)

            def attn_norm(b, hh):
                base = 64 * (hh % 2)
                j = hh // 2
                ybuf, r64 = state.pop((b, hh, "n"))
                nc.vector.tensor_mul(
                    out=y_t[base:base + 64, j, T * b:T * (b + 1)],
                    in0=ybuf, in1=r64)

            # ---------------- output projection ----------------
            def wo_evac(ps, tg, nh, out_f):
                ob = o_pool.tile([PPART, T], DT)
                nc.vector.tensor_add(
                    out=ob, in0=ps, in1=bo_bc[:, 512 * nh:512 * (nh + 1)])
                nc.sync.dma_start(
                    out=out_f[128 * tg:128 * (tg + 1),
                              512 * nh:512 * (nh + 1)], in_=ob)

            def wo_span(ci, ko_lo, ko_hi, sps, out_f, tag="wo"):
                tg, nh = ci // 2, ci % 2
                if ko_lo == 0:
                    state[("wo", ci)] = sps.tile([PPART, T], DT, tag=tag,
                                                 name="wops")
                ps = state[("wo", ci)]
                for ko in range(ko_lo, ko_hi + 1):
                    nc.tensor.matmul(
                        ps,
                        lhsT=y_t[:, ko, 128 * tg:128 * (tg + 1)],
                        rhs=wo_b[ko][:, 512 * nh:512 * (nh + 1)],
                        start=(ko == 0), stop=(ko == 7))
                if ko_hi == 7:
                    state.pop(("wo", ci))
                    wo_evac(ps, tg, nh, out_f)

            ATTN_N = int(os.environ.get("ATTN_N", "8"))

            def attn_phase(b, qkv, sps, yps, filler, tail1, tail2):
                q_t, k_t, v_t = qkv["q"], qkv["k"], qkv["v"]
                # pair 0's S+exp was pre-emitted in the projection scope.
                # norm(j-1) is emitted BEFORE attn_AV(j) so the 2-deep psy
                # ring reuse is legal (readers precede the overwriting
                # writer in emission order).
                for j in range(ATTN_N):
                    # S(j+1) first: it feeds exp(j+1), the ACT pacer --
                    # don't queue filler matmuls ahead of it in PE order
                    if j + 1 < 8:
                        attn_S(b, j + 1, q_t, k_t, sps)
                    filler(j)
                    if j >= 1:
                        attn_norm(b, 2 * (j - 1))
                        attn_norm(b, 2 * (j - 1) + 1)
                    attn_AV(b, j, v_t, yps)
                tail1()
                if ATTN_N == 8:
                    attn_norm(b, 14)
                    attn_norm(b, 15)
                elif ATTN_N >= 1:
                    attn_norm(b, 2 * (ATTN_N - 1))
                    attn_norm(b, 2 * (ATTN_N - 1) + 1)
                tail2()

            # ---------------- schedule ----------------
            for rep in range(reps):
              out_f = out_fs[rep]
              sfx = str(rep)
              # startup: x(b0) in, w8 Q-section casts, transposes w/ dual
              # evac (bf16 + fp8)
              with tc.tile_pool(name="ps0" + sfx, bufs=2,
                                space="PSUM") as trps:
                xb0 = xb_pool.tile([PPART, 8, T], BF, tag="xb", bufs=2)
                x80 = (x8_pool.tile([PPART, 8, T], F8, tag="x8", bufs=2,
                                   name="x80") if QK_DR else xb0)
                xns0 = [start_xn_dma(0, to, split=True) for to in range(4)]
                if rep == 0:
                    load_w8(0)
                    load_biases()
                    load_w8(1)
                for to in range(4):
                    for fg in range(2):
                        transpose_fg(xb0, xns0[to], to, fg, trps,
                                     x8_t=x80 if QK_DR else None)
                if rep == 0:
                    load_wv()
                # b1 transposes also at startup (PE is otherwise idle
                # under the weight-DMA wait)
                xb1 = xb_pool.tile([PPART, 8, T], BF, tag="xb", bufs=2)
                x81 = (x8_pool.tile([PPART, 8, T], F8, tag="x8", bufs=2,
                                   name="x81") if QK_DR else xb1)
                xns1 = [start_xn_dma(1, to) for to in range(4)]
                for to in range(4):
                    for fg in range(2):
                        transpose_fg(xb1, xns1[to], to, fg, trps,
                                     x8_t=x81 if QK_DR else None)
              # Q(b0), K(b0) DoubleRow; V(b0) bf16; pre-emit S(b0, pair0)
              with tc.tile_pool(name="qps0" + sfx, bufs=1,
                                space="PSUM") as qps:
                qkv0 = alloc_qkv("qkv")
                for fo in range(8):
                    qk_fo(0, x80, qkv0["q"], fo, qps)
                for fo in range(8):
                    qk_fo(1, x80, qkv0["k"], fo, qps)
                # S(pair0) needs only the fo=0 q/k blocks -- emit it before
                # the V section so its exp overlaps V's PE work
                if PHASES >= 2:
                    attn_S(0, 0, qkv0["q"], qkv0["k"], qps, tag="s0")
                for nh in range(2):
                    for to in range(4):
                        v_block(xb0, qkv0["v"], to, nh, qps)
                    if PHASES == 2:
                        pch0 = state.pop((0, 0))
                        ob2 = o_pool.tile([PPART, T], DT)
                        nc.vector.tensor_copy(out=ob2, in_=pch0[:, 0:512])
                        nc.sync.dma_start(out=out_f[128:256, 0:512], in_=ob2)
              if PHASES < 2 or PHASES == 2:
                ob = o_pool.tile([PPART, T], DT)
                nc.vector.tensor_copy(out=ob, in_=qkv0["q"][:, 0, :])
                nc.sync.dma_start(out=out_f[0:128, 0:512], in_=ob)
                continue
              # attention(b0); fillers run Q(b1) and V(b1) from resident
              # weights (full-row-closed groups may share the one filler
              # bank back-to-back; S/AV matmuls separate the slots)
              with (
                tc.tile_pool(name="aps0" + sfx, bufs=1, space="PSUM") as sps,
                tc.tile_pool(name="ay0" + sfx, bufs=2, space="PSUM") as yps,
                tc.tile_pool(name="atr0" + sfx, bufs=1, space="PSUM") as trps,
              ):
                q1 = alloc_qkv("qv")
                if rep == 0:
                    load_wo()

                def filler_b0(j):
                    qk_fo(0, x81, q1["q"], j, trps, tag="tr", bufs=1,
                          evac="dve")
                    v_block(xb1, q1["v"], j % 4, j // 4, trps, tag="tr",
                            bufs=1)

                attn_phase(0, qkv0, sps, yps, filler_b0,
                           lambda: None, lambda: None)
              if PHASES < 3:
                ob = o_pool.tile([PPART, T], DT)
                nc.vector.tensor_copy(out=ob, in_=y_t[:, 0, 0:512])
                nc.sync.dma_start(out=out_f[0:128, 0:512], in_=ob)
                continue
              # K(b1) from resident weights; pre-emit S(b1, pair0)
              with tc.tile_pool(name="qps1" + sfx, bufs=1,
                                space="PSUM") as qps:
                kv1 = alloc_qkv("k")
                qk_fo(1, x81, kv1["k"], 0, qps)
                qkv1 = {"q": q1["q"], "k": kv1["k"], "v": q1["v"]}
                if PHASES >= 4:
                    attn_S(1, 0, qkv1["q"], qkv1["k"], qps, tag="s0")
                for fo in range(1, 8):
                    qk_fo(1, x81, kv1["k"], fo, qps)
              if PHASES < 4:
                ob = o_pool.tile([PPART, T], DT)
                nc.vector.tensor_copy(out=ob, in_=kv1["k"][:, 0, :])
                nc.sync.dma_start(out=out_f[0:128, 0:512], in_=ob)
                continue
              # attention(b1); fillers run W_O spans; tail finishes W_O
              with (
                tc.tile_pool(name="aps1" + sfx, bufs=1, space="PSUM") as sps,
                tc.tile_pool(name="ay1" + sfx, bufs=2, space="PSUM") as yps,
                tc.tile_pool(name="awo" + sfx, bufs=1, space="PSUM") as wps,
              ):
                def filler_b1(j):
                    wo_span(j, 0, 7, wps, out_f)

                def tail1_b1():
                    # chunks with tg>=4 read b1 tokens; heads 14/15 are
                    # normalized after tail1, so hold back their ko=7 span
                    wo_span(8, 0, 6, wps, out_f)

                def tail2_b1():
                    wo_span(8, 7, 7, wps, out_f)
                    for ci in range(9, 16):
                        tag = "wo" if ci % 3 == 0 else "y"
                        pool = wps if tag == "wo" else yps
                        wo_span(ci, 0, 7, pool, out_f, tag=tag)

                attn_phase(1, qkv1, sps, yps, filler_b1, tail1_b1, tail2_b1)

    nc.compile()
    return nc


def _get_program(reps=1, phases="xqaw"):
    key = f"nc{reps}"
    if key not in _CACHE:
        _CACHE[key] = _build_program(reps)
    return _CACHE[key]


def kernel(x, w_qkv, b_qkv, w_o, b_o):
    from concourse.bass_utils import run_bass_kernel_spmd

    nc = _get_program()
    x = np.ascontiguousarray(x, dtype=np.float32)
    in_maps = []
    for c in range(NCORES):
        in_maps.append({
            "x": x[BL * c:BL * (c + 1)],
            "w_qkv": np.asarray(w_qkv, dtype=np.float32),
            "b_qkv": np.asarray(b_qkv, dtype=np.float32),
            "w_o": np.asarray(w_o, dtype=np.float32),
            "b_o": np.asarray(b_o, dtype=np.float32),
        })
    res = run_bass_kernel_spmd(nc, in_maps, list(range(NCORES)))
    return np.concatenate([res.results[c]["out"] for c in range(NCORES)], axis=0)


# revision 51
# speedup vs baseline: 1.0599x; 1.0599x over previous
"""Multi-head causal attention (B=16, T=512, D=1024, H=16) on 8 TRN2 cores.

Sharding: data-parallel over batch (2 batches per core), weights replicated.

Design (vs the previous kernel):
- Weights are loaded from HBM ONCE per invocation into reduced-precision
  resident SBUF tiles (fp8e4 DoubleRow layout for Q/K via SWDGE cast-DMA,
  bf16 for V/W_O): HBM traffic 36MB -> 24MB, and the b1 phases run with
  zero weight DMA.
- Q/K projections are fp8e4 DoubleRow matmuls (2x contraction per cycle;
  HW-validated ~2x vs bf16, rel err 4.3e-3 vs the 2e-2 gate -- softmax
  damps Q/K quantization; V/W_O in fp8 would fail at ~3.8e-2, so those
  stay bf16).
- Attention processes HEAD PAIRS (heads 2j/2j+1 on feature partitions
  0-63/64-127): the K=64 S^T matmuls alternate PE row groups so the two
  heads' matmuls run concurrently (HW-validated 2.28x incl. LDWEIGHTS
  overlap). The pair's 8 causal chunks pack into 5 PSUM banks
  (b0:A-kt0 | b1:A-kt1+A-kt3 | b2:A-kt2+B-kt2 | b3:B-kt0 | b4:B-kt1+B-kt3).
- HW hazard found by bisection: back-to-back matmul groups into the SAME
  psum bank with K=64 row-tiled closers crash the device (group-boundary
  bank revisit). Rule: separate same-bank groups with other-bank matmuls
  AND close shared-bank groups with a full-row (K=128) matmul. The causal
  mask matmuls (ident @ maskT, -1e30 additive) serve as those closers for
  kt1/kt2/kt3; kt0 (unshared, single-group) is masked post-exp by a DVE
  lower-triangular multiply.
- exp(S/32) evacuates per bank on ACT (5 ops/pair) so the next pair's S
  matmuls unblock per-bank instead of waiting a fused whole-tile read.
- Softmax normalize: ones column in V gives row sums in psy[64]; psy is
  evacuated to SBUF (bf16) immediately after AV so the psum bank frees
  after two fast DVE reads (copy + reciprocal) instead of being held
  through broadcast+normalize; gpsimd partition_broadcast of the
  reciprocal row, then a 2x-rate bf16 DVE multiply into y_t.
- Schedule: both batches' x^T transposes run at startup under the weight
  DMA; Q(b1)+V(b1) fill PE during attention(b0) (one full-row-closed
  group per pair slot in the shared filler bank); K(b1) between phases;
  full W_O chunks fill attention(b1); remaining W_O chunks drain in the
  tail rotating over 3 psum banks. Each batch's S(pair0) is pre-emitted
  as soon as its fo=0 q/k blocks exist (before the V section / rest of
  K), so the first exp overlaps the remaining projection matmuls.
Measured (same-day, conditions-matched differential bench): 135-204us
median depending on device contention (min-quartile 81us in quiet
windows) vs 258-280us for the previous kernel, rel err 4.1e-3.
"""

import os
import sys

sys.path.insert(0, "/opt/trn_rl_repo")

import numpy as np

QK_DR = os.environ.get("QK_DR", "1") == "1"
CAST_DMA = os.environ.get("CAST_DMA", "1") == "1"
FUSED_EXP = os.environ.get("FUSED_EXP", "0") == "1"
MULTIBANK = os.environ.get("MULTIBANK", "0") == "1"
MASKS = os.environ.get("MASKS", "1") == "1"
PHASES = int(os.environ.get("PHASES", "4"))

B, T, D = 16, 512, 1024
H = 16
HD = D // H          # 64
NCORES = 8
BL = B // NCORES     # 2 local batches per core
PPART = 128
VW = HD + 1          # 65: head dim + ones column for row sums

_CACHE = {}


def _build_program(reps=1):
    import concourse.bass as bass
    import concourse.tile as tile
    from concourse import bacc, mybir
    from concourse.masks import make_identity

    DT = mybir.dt.float32
    R = mybir.dt.float32r
    BF = mybir.dt.bfloat16
    F8 = mybir.dt.float8e4
    ACTF = mybir.ActivationFunctionType
    DR = mybir.MatmulPerfMode.DoubleRow

    nc = bacc.Bacc("TRN2", target_bir_lowering=False, debug=False,
                   num_devices=NCORES)

    x_d = nc.dram_tensor("x", [BL, T, D], DT, kind="ExternalInput").ap()
    wqkv_d = nc.dram_tensor("w_qkv", [D, 3 * D], DT, kind="ExternalInput").ap()
    bqkv_d = nc.dram_tensor("b_qkv", [3 * D], DT, kind="ExternalInput").ap()
    wo_d = nc.dram_tensor("w_o", [D, D], DT, kind="ExternalInput").ap()
    bo_d = nc.dram_tensor("b_o", [D], DT, kind="ExternalInput").ap()
    out_d = nc.dram_tensor("out", [BL, T, D], DT, kind="ExternalOutput").ap()

    x_f = x_d.flatten_outer_dims()      # [1024, 1024] tokens x features
    out_fs = [out_d.flatten_outer_dims()]
    for r in range(1, reps):
        scr = nc.dram_tensor(f"scratch{r}", [BL, T, D], DT).ap()
        out_fs.append(scr.flatten_outer_dims())

    def f32r(ap):
        return ap.bitcast(R)

    with tile.TileContext(nc) as tc:
        with (
            tc.tile_pool(name="consts", bufs=1) as consts,
            tc.tile_pool(name="w8", bufs=1) as w8_pool,
            tc.tile_pool(name="wvb", bufs=1) as wv_pool,
            tc.tile_pool(name="wob", bufs=1) as wo_pool,
            tc.tile_pool(name="y", bufs=1) as y_pool,
            tc.tile_pool(name="xb", bufs=1) as xb_pool,
            tc.tile_pool(name="x8", bufs=1) as x8_pool,
            tc.tile_pool(name="qkv", bufs=1) as qkv_pool,
            tc.tile_pool(name="xn", bufs=4) as xn_pool,
            tc.tile_pool(name="pp", bufs=3) as p_pool,
            tc.tile_pool(name="ss", bufs=4) as s_pool,
            tc.tile_pool(name="ob", bufs=3) as o_pool,
        ):
            # ---------------- constants ----------------
            ident_f = consts.tile([PPART, PPART], DT)
            make_identity(nc, ident_f)
            ident = consts.tile([PPART, PPART], R)
            nc.vector.tensor_copy(out=ident, in_=ident_f)

            # lower-tri 0/1 bf16 mask for post-exp causal zeroing of the
            # diagonal blocks of P^T: entry (k=p, q=c) kept iff c >= p
            trimask_f = consts.tile([PPART, PPART], DT)
            nc.vector.memset(trimask_f, 1.0)
            nc.gpsimd.affine_select(
                out=trimask_f, in_=trimask_f,
                compare_op=mybir.AluOpType.is_ge, fill=0.0,
                base=0, pattern=[[1, PPART]], channel_multiplier=-1,
            )
            trimask = consts.tile([PPART, PPART], BF)
            nc.vector.tensor_copy(out=trimask, in_=trimask_f)

            # additive causal mask for the PE group-closer matmuls:
            # (k=i, q=j) kept (0.0) where j - i >= 0, else -1e30
            maskneg = consts.tile([PPART, PPART], DT)
            nc.vector.memset(maskneg, 0.0)
            nc.gpsimd.affine_select(
                out=maskneg, in_=maskneg,
                compare_op=mybir.AluOpType.is_ge, fill=-1e30,
                base=0, pattern=[[1, PPART]], channel_multiplier=-1,
            )
            maskT_b = consts.tile([PPART, PPART], BF)
            nc.vector.tensor_copy(out=maskT_b, in_=maskneg)
            ident_b = consts.tile([PPART, PPART], BF)
            nc.vector.tensor_copy(out=ident_b, in_=ident_f)
            zcol = consts.tile([PPART, 1], BF)
            nc.vector.memset(zcol, 0.0)

            # qkv bias, feature-major columns: bcol[p, c] = b_qkv[128c + p]
            bcol = consts.tile([PPART, 24], DT)
            brow = consts.tile([1, D], DT)
            bv_bc = consts.tile([PPART, D], DT)
            bo_bc = consts.tile([PPART, D], DT)

            def load_biases():
                nc.sync.dma_start(
                    out=bcol, in_=bqkv_d.rearrange("(c p) -> p c", p=PPART))
                nc.sync.dma_start(
                    out=brow,
                    in_=bqkv_d[2 * D:3 * D].rearrange("(a f) -> a f", a=1))
                nc.gpsimd.partition_broadcast(bv_bc, brow, channels=PPART)
                nc.sync.dma_start(
                    out=brow, in_=bo_d.rearrange("(a f) -> a f", a=1))
                nc.gpsimd.partition_broadcast(bo_bc, brow, channels=PPART)

            # ---------------- resident weights ----------------
            # fp8e4 DoubleRow layout for Q,K: w8[sec][c][p, i, f] =
            # w_qkv[256c + 128i + p, 1024 sec + f]   (unscaled; validated)
            w8 = {}

            def _cast_load(dst, src_ap, flat=None):
                # HBM fp32 -> reduced-precision SBUF tile: SWDGE cast DMA,
                # or (fallback) HWDGE fp32 stage + DVE convert
                if CAST_DMA:
                    nc.gpsimd.dma_start(out=dst, in_=src_ap)
                else:
                    dflat = flat if flat is not None else dst
                    stage = consts.tile([PPART, 2048], DT, tag="wstage",
                                        bufs=1, name="wstage")
                    sz = dflat.free_size()
                    nc.sync.dma_start(out=stage[:, :sz], in_=src_ap)
                    nc.vector.tensor_copy(out=dflat, in_=stage[:, :sz])

            def load_w8(sec):
                tiles = []
                for c in range(4):
                    t = w8_pool.tile([PPART, 2, 1024], F8 if QK_DR else BF,
                                     name=f"w8_{sec}_{c}")
                    _cast_load(
                        t,
                        wqkv_d[256 * c:256 * (c + 1),
                               1024 * sec:1024 * (sec + 1)]
                        .rearrange("(i p) f -> p i f", p=PPART),
                        flat=t.rearrange("p a b -> p (a b)"))
                    tiles.append(t)
                w8[sec] = tiles

            wv_b = []

            def load_wv():
                for ko in range(8):
                    t = wv_pool.tile([PPART, 1024], BF, name=f"wv_{ko}")
                    _cast_load(t, wqkv_d[128 * ko:128 * (ko + 1), 2048:3072])
                    wv_b.append(t)

            wo_b = []

            def load_wo():
                for ko in range(8):
                    t = wo_pool.tile([PPART, 1024], BF, name=f"wo_{ko}")
                    _cast_load(t, wo_d[128 * ko:128 * (ko + 1), :])
                    wo_b.append(t)

            y_t = y_pool.tile([PPART, 8, BL * T], BF)  # [128, 8, 1024]

            # ---------------- x load / transpose / convert ----------------
            def start_xn_dma(b, to, split=False):
                xn = xn_pool.tile([PPART, D], R)
                src = x_f[T * b + 128 * to:T * b + 128 * (to + 1), :]
                if split:
                    for c in range(2):
                        nc.sync.dma_start(
                            out=xn[:, 512 * c:512 * (c + 1)],
                            in_=f32r(src[:, 512 * c:512 * (c + 1)]))
                else:
                    nc.sync.dma_start(out=xn, in_=f32r(src))
                return xn

            def transpose_fg(xb_t, xn, to, fg, trps, x8_t=None):
                # 4 transposes -> one psum tile -> DVE evac to bf16 xb_t
                # (and optionally a second DVE evac to fp8 x8_t for b0)
                pst_d = trps.tile([PPART, 4 * PPART], DT, tag="tr")
                pst = pst_d.bitcast(R).rearrange("p (f q) -> p f q", f=4)
                for fi in range(4):
                    fo = 4 * fg + fi
                    nc.tensor.transpose(
                        pst[:, fi, :], xn[:, 128 * fo:128 * (fo + 1)], ident)
                dst = xb_t[:, 4 * fg:4 * (fg + 1), 128 * to:128 * (to + 1)]
                nc.vector.tensor_copy(out=dst, in_=pst_d)
                if x8_t is not None:
                    dst8 = x8_t[:, 4 * fg:4 * (fg + 1),
                                128 * to:128 * (to + 1)]
                    nc.vector.tensor_copy(out=dst8, in_=pst_d)

            def x8_cast_dma(x8_t, xb_t, c):
                # SWDGE cast-copy bf16 -> fp8 of fo pair block c (b1 path)
                if not QK_DR:
                    return
                if CAST_DMA:
                    nc.gpsimd.dma_start(
                        out=x8_t[:, 2 * c:2 * (c + 1), :],
                        in_=xb_t[:, 2 * c:2 * (c + 1), :])
                else:
                    nc.vector.tensor_copy(
                        out=x8_t[:, 2 * c:2 * (c + 1), :],
                        in_=xb_t[:, 2 * c:2 * (c + 1), :])

            # ---------------- projections ----------------
            def alloc_qkv(which):
                t = {}
                if "q" in which:
                    t["q"] = qkv_pool.tile([PPART, 8, T], BF, tag="q",
                                           name="q_t", bufs=2)
                if "k" in which:
                    t["k"] = qkv_pool.tile([PPART, 8, T], BF, tag="k",
                                           name="k_t", bufs=2)
                if "v" in which:
                    v_t = qkv_pool.tile([PPART, 4, H * VW], BF, tag="v",
                                        bufs=2)
                    ones = v_t.rearrange("p t (h c) -> p t h c",
                                         c=VW)[:, :, :, HD:]
                    nc.vector.memset(ones, 1.0)
                    t["v"] = v_t
                return t

            def qk_fo(sec, x8_t, dst, fo, qps, tag="qkv", bufs=3,
                      evac="act"):
                # one output block of the Q or K projection: 4 DoubleRow
                # matmuls (256-feature chunks) into one psum, ACT evac+bias
                ps = qps.tile([PPART, T], DT, tag=tag, bufs=bufs,
                              name=f"qk{sec}{fo}")
                if QK_DR:
                    for c in range(4):
                        nc.tensor.matmul(
                            ps,
                            lhsT=w8[sec][c][:, :, 128 * fo:128 * (fo + 1)],
                            rhs=x8_t[:, 2 * c:2 * (c + 1), :],
                            perf_mode=DR,
                            start=(c == 0), stop=(c == 3))
                else:
                    for c in range(4):
                        for i in range(2):
                            nc.tensor.matmul(
                                ps,
                                lhsT=w8[sec][c][:, i,
                                                128 * fo:128 * (fo + 1)],
                                rhs=x8_t[:, 2 * c + i, :],
                                start=(c == 0 and i == 0),
                                stop=(c == 3 and i == 1))
                if evac == "act":
                    nc.scalar.activation(
                        out=dst[:, fo, :], in_=ps, func=ACTF.Identity,
                        bias=bcol[:, 8 * sec + fo:8 * sec + fo + 1])
                else:
                    # DVE evac keeps ACT free for the attention exps
                    nc.vector.tensor_scalar_add(
                        out=dst[:, fo, :], in0=ps,
                        scalar1=bcol[:, 8 * sec + fo:8 * sec + fo + 1])

            def v_block(xb_t, v_t, to, nh, qps, tag="qkv", bufs=3):
                # one V psum block: 8 bf16 matmuls, DVE evac + bias
                ps = qps.tile([PPART, T], DT, tag=tag, bufs=bufs,
                              name=f"v{to}{nh}")
                for ko in range(8):
                    nc.tensor.matmul(
                        ps,
                        lhsT=xb_t[:, ko, 128 * to:128 * (to + 1)],
                        rhs=wv_b[ko][:, 512 * nh:512 * (nh + 1)],
                        start=(ko == 0), stop=(ko == 7))
                vv = v_t[:, to, 8 * VW * nh:8 * VW * (nh + 1)]
                vv = vv.rearrange("p (h c) -> p h c", c=VW)[:, :, :HD]
                nc.vector.tensor_add(
                    out=vv, in0=ps, in1=bv_bc[:, 512 * nh:512 * (nh + 1)])

            # --------------- attention (head pairs) ---------------
            # pair j = heads (2j, 2j+1) at feature partitions 0-63 / 64-127.
            # S^T chunks land in one 5-bank psum tile s5 [128, 5, 512]:
            #  bank0: A kt0 (q0-512)   bank1: A kt1 (q128-512) + A kt3
            #  bank2: A kt2 (q256-512) + B kt2    bank3: B kt0
            #  bank4: B kt1 + B kt3
            # -> ONE exp op over 2560 cols -> pch bf16 [128, 2560]
            state = {}
            SL_A = {0: (0, 0, 512), 1: (1, 0, 384), 2: (2, 0, 256),
                    3: (1, 384, 128)}
            SL_B = {0: (3, 0, 512), 1: (4, 0, 384), 2: (2, 256, 256),
                    3: (4, 384, 128)}
            # pch col offsets of each chunk (bank*512 + off)
            PCH_A = {kt: SL_A[kt][0] * 512 + SL_A[kt][1] for kt in range(4)}
            PCH_B = {kt: SL_B[kt][0] * 512 + SL_B[kt][1] for kt in range(4)}
            # diagonal 128-col blocks to mask (pch col offsets)
            DIAG = [PCH_A[0], PCH_A[1], PCH_A[3], PCH_A[2],
                    PCH_B[0], PCH_B[1], PCH_B[3], PCH_B[2]]

            def attn_S(b, j, q_t, k_t, sps, tag="s", bufs=1):
                if MULTIBANK:
                    s5 = sps.tile([PPART, 5, 512], DT, tag=tag, bufs=bufs,
                                  name="s5")
                    banks = [s5[:, i, :] for i in range(5)]
                else:
                    banks = [sps.tile([PPART, 512], DT, tag=tag,
                                      bufs=5 * bufs, name=f"s5_{i}")
                             for i in range(5)]

                def smm(kt, h, start, stop):
                    base = 64 * h
                    bank, off, nq = (SL_A if h == 0 else SL_B)[kt]
                    nc.tensor.matmul(
                        banks[bank][:, off:off + nq],
                        lhsT=k_t[base:base + 64, j,
                                 128 * kt:128 * (kt + 1)],
                        rhs=q_t[base:base + 64, j, 128 * kt:],
                        start=start, stop=stop, skip_group_check=True)

                def mmask(kt, h):
                    # K=128 causal-mask matmul: applies the additive mask
                    # to the chunk's diagonal block AND closes the psum
                    # group (shared-bank groups not closed by a full-row
                    # matmul hazard the PE->PSUM path)
                    bank, off, nq = (SL_A if h == 0 else SL_B)[kt]
                    nc.tensor.matmul(
                        banks[bank][:, off:off + 128],
                        lhsT=ident_b, rhs=maskT_b,
                        start=False, stop=True, skip_group_check=True)

                # order avoids consecutive same-bank writes across group
                # boundaries while alternating A/B row groups for PE
                # concurrency; kt0 (unshared banks) closes immediately
                # and its diagonal is masked post-exp on DVE
                smm(0, 0, True, True)
                smm(0, 1, True, True)
                smm(1, 0, True, False)
                smm(1, 1, True, False)
                mmask(1, 0)
                mmask(1, 1)
                smm(2, 0, True, False)
                mmask(2, 0)
                smm(3, 0, True, False)
                smm(3, 1, True, False)
                mmask(3, 0)
                mmask(3, 1)
                smm(2, 1, True, False)
                mmask(2, 1)
                pch = p_pool.tile([PPART, 5 * 512], BF, tag="P")
                if FUSED_EXP and MULTIBANK:
                    nc.scalar.activation(
                        out=pch, in_=s5.rearrange("p a b -> p (a b)"),
                        func=ACTF.Exp, scale=1.0 / 32.0)
                else:
                    for bank in range(5):
                        nc.scalar.activation(
                            out=pch[:, 512 * bank:512 * (bank + 1)],
                            in_=banks[bank],
                            func=ACTF.Exp, scale=1.0 / 32.0)
                # only the kt0 diagonals need DVE masking (kt1-3 were
                # masked by the PE group-closer matmuls pre-exp)
                for off in (PCH_A[0], PCH_B[0]):
                    nc.vector.tensor_mul(
                        out=pch[:, off:off + 128],
                        in0=pch[:, off:off + 128], in1=trimask)
                state[(b, j)] = pch

            def attn_AV(b, j, v_t, yps):
                pch = state.pop((b, j))
                for h in range(2):
                    hh = 2 * j + h
                    pc = PCH_A if h == 0 else PCH_B
                    psy = yps.tile([PPART, T], DT, tag="y", name=f"psy{h}")
                    for kt in range(4):
                        nc.tensor.matmul(
                            psy[:VW, 128 * kt:],
                            lhsT=v_t[:, kt, VW * hh:VW * (hh + 1)],
                            rhs=pch[:, pc[kt]:pc[kt] + (512 - 128 * kt)],
                            start=(kt == 0), stop=(kt == 3))
                    # evacuate psy to SBUF right away so the psum bank
                    # frees after two fast DVE reads (copy + recip) instead
                    # of being held through broadcast+normalize
                    ybuf = s_pool.tile([HD, T], BF, tag="yb", bufs=4,
                                       name="ybuf")
                    nc.vector.tensor_copy(out=ybuf, in_=psy[:HD, :])
                    r_row = s_pool.tile([1, T], BF, tag="r1")
                    with nc.allow_low_precision(reason="bf16 softmax recip"):
                        nc.vector.reciprocal(r_row, psy[HD:HD + 1, :])
                    r64 = s_pool.tile([HD, T], BF, tag="r64", bufs=4)
                    nc.gpsimd.partition_broadcast(r64, r_row, channels=HD)
                    state[(b, hh, "n")] = (ybuf, r64)

            def attn_norm(b, hh):
                base = 64 * (hh % 2)
                j = hh // 2
                ybuf, r64 = state.pop((b, hh, "n"))
                nc.vector.tensor_mul(
                    out=y_t[base:base + 64, j, T * b:T * (b + 1)],
                    in0=ybuf, in1=r64)

            # ---------------- output projection ----------------
            def wo_evac(ps, tg, nh, out_f):
                ob = o_pool.tile([PPART, T], DT)
                nc.vector.tensor_add(
                    out=ob, in0=ps, in1=bo_bc[:, 512 * nh:512 * (nh + 1)])
                nc.sync.dma_start(
                    out=out_f[128 * tg:128 * (tg + 1),
                              512 * nh:512 * (nh + 1)], in_=ob)

            def wo_span(ci, ko_lo, ko_hi, sps, out_f, tag="wo"):
                tg, nh = ci // 2, ci % 2
                if ko_lo == 0:
                    state[("wo", ci)] = sps.tile([PPART, T], DT, tag=tag,
                                                 name="wops")
                ps = state[("wo", ci)]
                for ko in range(ko_lo, ko_hi + 1):
                    nc.tensor.matmul(
                        ps,
                        lhsT=y_t[:, ko, 128 * tg:128 * (tg + 1)],
                        rhs=wo_b[ko][:, 512 * nh:512 * (nh + 1)],
                        start=(ko == 0), stop=(ko == 7))
                if ko_hi == 7:
                    state.pop(("wo", ci))
                    wo_evac(ps, tg, nh, out_f)

            ATTN_N = int(os.environ.get("ATTN_N", "8"))

            def attn_phase(b, qkv, sps, yps, filler, tail1, tail2):
                q_t, k_t, v_t = qkv["q"], qkv["k"], qkv["v"]
                # pair 0's S+exp was pre-emitted in the projection scope.
                # norm(j-1) is emitted BEFORE attn_AV(j) so the 2-deep psy
                # ring reuse is legal (readers precede the overwriting
                # writer in emission order).
                for j in range(ATTN_N):
                    filler(j)
                    if j >= 1:
                        attn_norm(b, 2 * (j - 1))
                        attn_norm(b, 2 * (j - 1) + 1)
                    if j + 1 < 8:
                        attn_S(b, j + 1, q_t, k_t, sps)
                    attn_AV(b, j, v_t, yps)
                tail1()
                if ATTN_N == 8:
                    attn_norm(b, 14)
                    attn_norm(b, 15)
                elif ATTN_N >= 1:
                    attn_norm(b, 2 * (ATTN_N - 1))
                    attn_norm(b, 2 * (ATTN_N - 1) + 1)
                tail2()

            # ---------------- schedule ----------------
            for rep in range(reps):
              out_f = out_fs[rep]
              sfx = str(rep)
              # startup: x(b0) in, w8 Q-section casts, transposes w/ dual
              # evac (bf16 + fp8)
              with tc.tile_pool(name="ps0" + sfx, bufs=2,
                                space="PSUM") as trps:
                xb0 = xb_pool.tile([PPART, 8, T], BF, tag="xb", bufs=2)
                x80 = (x8_pool.tile([PPART, 8, T], F8, tag="x8", bufs=2,
                                   name="x80") if QK_DR else xb0)
                xns0 = [start_xn_dma(0, to, split=True) for to in range(4)]
                if rep == 0:
                    load_w8(0)
                    load_biases()
                    load_w8(1)
                for to in range(4):
                    for fg in range(2):
                        transpose_fg(xb0, xns0[to], to, fg, trps,
                                     x8_t=x80 if QK_DR else None)
                if rep == 0:
                    load_wv()
                # b1 transposes also at startup (PE is otherwise idle
                # under the weight-DMA wait)
                xb1 = xb_pool.tile([PPART, 8, T], BF, tag="xb", bufs=2)
                x81 = (x8_pool.tile([PPART, 8, T], F8, tag="x8", bufs=2,
                                   name="x81") if QK_DR else xb1)
                xns1 = [start_xn_dma(1, to) for to in range(4)]
                for to in range(4):
                    for fg in range(2):
                        transpose_fg(xb1, xns1[to], to, fg, trps,
                                     x8_t=x81 if QK_DR else None)
              # Q(b0), K(b0) DoubleRow; V(b0) bf16; pre-emit S(b0, pair0)
              with tc.tile_pool(name="qps0" + sfx, bufs=1,
                                space="PSUM") as qps:
                qkv0 = alloc_qkv("qkv")
                for fo in range(8):
                    qk_fo(0, x80, qkv0["q"], fo, qps)
                for fo in range(8):
                    qk_fo(1, x80, qkv0["k"], fo, qps)
                # S(pair0) needs only the fo=0 q/k blocks -- emit it before
                # the V section so its exp overlaps V's PE work
                if PHASES >= 2:
                    attn_S(0, 0, qkv0["q"], qkv0["k"], qps, tag="s0")
                for nh in range(2):
                    for to in range(4):
                        v_block(xb0, qkv0["v"], to, nh, qps)
                    if PHASES == 2:
                        pch0 = state.pop((0, 0))
                        ob2 = o_pool.tile([PPART, T], DT)
                        nc.vector.tensor_copy(out=ob2, in_=pch0[:, 0:512])
                        nc.sync.dma_start(out=out_f[128:256, 0:512], in_=ob2)
              if PHASES < 2 or PHASES == 2:
                ob = o_pool.tile([PPART, T], DT)
                nc.vector.tensor_copy(out=ob, in_=qkv0["q"][:, 0, :])
                nc.sync.dma_start(out=out_f[0:128, 0:512], in_=ob)
                continue
              # attention(b0); fillers run Q(b1) and V(b1) from resident
              # weights (full-row-closed groups may share the one filler
              # bank back-to-back; S/AV matmuls separate the slots)
              with (
                tc.tile_pool(name="aps0" + sfx, bufs=1, space="PSUM") as sps,
                tc.tile_pool(name="ay0" + sfx, bufs=2, space="PSUM") as yps,
                tc.tile_pool(name="atr0" + sfx, bufs=1, space="PSUM") as trps,
              ):
                q1 = alloc_qkv("qv")
                if rep == 0:
                    load_wo()

                def filler_b0(j):
                    qk_fo(0, x81, q1["q"], j, trps, tag="tr", bufs=1,
                          evac="dve")
                    v_block(xb1, q1["v"], j % 4, j // 4, trps, tag="tr",
                            bufs=1)

                attn_phase(0, qkv0, sps, yps, filler_b0,
                           lambda: None, lambda: None)
              if PHASES < 3:
                ob = o_pool.tile([PPART, T], DT)
                nc.vector.tensor_copy(out=ob, in_=y_t[:, 0, 0:512])
                nc.sync.dma_start(out=out_f[0:128, 0:512], in_=ob)
                continue
              # K(b1) from resident weights; pre-emit S(b1, pair0)
              with tc.tile_pool(name="qps1" + sfx, bufs=1,
                                space="PSUM") as qps:
                kv1 = alloc_qkv("k")
                qk_fo(1, x81, kv1["k"], 0, qps)
                qkv1 = {"q": q1["q"], "k": kv1["k"], "v": q1["v"]}
                if PHASES >= 4:
                    attn_S(1, 0, qkv1["q"], qkv1["k"], qps, tag="s0")
                for fo in range(1, 8):
                    qk_fo(1, x81, kv1["k"], fo, qps)
              if PHASES < 4:
                ob = o_pool.tile([PPART, T], DT)
                nc.vector.tensor_copy(out=ob, in_=kv1["k"][:, 0, :])
                nc.sync.dma_start(out=out_f[0:128, 0:512], in_=ob)
                continue
              # attention(b1); fillers run W_O spans; tail finishes W_O
              with (
                tc.tile_pool(name="aps1" + sfx, bufs=1, space="PSUM") as sps,
                tc.tile_pool(name="ay1" + sfx, bufs=2, space="PSUM") as yps,
                tc.tile_pool(name="awo" + sfx, bufs=1, space="PSUM") as wps,
              ):
                def filler_b1(j):
                    wo_span(j, 0, 7, wps, out_f)

                def tail1_b1():
                    # chunks with tg>=4 read b1 tokens; heads 14/15 are
                    # normalized after tail1, so hold back their ko=7 span
                    wo_span(8, 0, 6, wps, out_f)

                def tail2_b1():
                    wo_span(8, 7, 7, wps, out_f)
                    for ci in range(9, 16):
                        tag = "wo" if ci % 3 == 0 else "y"
                        pool = wps if tag == "wo" else yps
                        wo_span(ci, 0, 7, pool, out_f, tag=tag)

                attn_phase(1, qkv1, sps, yps, filler_b1, tail1_b1, tail2_b1)

    nc.compile()
    return nc


def _get_program(reps=1, phases="xqaw"):
    key = f"nc{reps}"
    if key not in _CACHE:
        _CACHE[key] = _build_program(reps)
    return _CACHE[key]


def kernel(x, w_qkv, b_qkv, w_o, b_o):
    from concourse.bass_utils import run_bass_kernel_spmd

    nc = _get_program()
    x = np.ascontiguousarray(x, dtype=np.float32)
    in_maps = []
    for c in range(NCORES):
        in_maps.append({
            "x": x[BL * c:BL * (c + 1)],
            "w_qkv": np.asarray(w_qkv, dtype=np.float32),
            "b_qkv": np.asarray(b_qkv, dtype=np.float32),
            "w_o": np.asarray(w_o, dtype=np.float32),
            "b_o": np.asarray(b_o, dtype=np.float32),
        })
    res = run_bass_kernel_spmd(nc, in_maps, list(range(NCORES)))
    return np.concatenate([res.results[c]["out"] for c in range(NCORES)], axis=0)
